# revision 19
# baseline (speedup 1.0000x reference)
"""Multi-head attention (B=4, S=2048, D=1024, H=16) on 8 trn2 NeuronCores.

Sharding: core c -> (batch b = c//2, head-half hh = c%2). Each core computes
attention for 8 heads of one batch element over the full sequence, plus the
partial output projection for its 512 feature rows of w_o. The host sums the
two partial projections per batch element (perfect 1/8 FLOP balance, no
collectives).

Per-core dataflow (fp16 matmul operands, fp32 PSUM accumulation):
  xT [1024, 2048] (host-transposed x[b], fp16)
  KT = wk.T @ x.T -> [512, 2048]      d-on-partitions, head-pair chunks
  QT = wq.T @ x.T -> zero-padded [128, 8, 2048]: head h occupies rows
       (h%2)*64..+64 of slot h, rest zeros, so every scores matmul streams a
       full K=128 contraction (half-K matmuls don't register as PE activity
       for the HAM clock gate and leave the PE throttled at 1.2 GHz)
  V   = x @ wv -> [2048, 8, 65] token-on-partitions with a ones column;
       the PV matmul then produces the softmax denominators for free in
       PSUM row 64
  Phase 2 is a 3-stage software pipeline over the 16 (pair, q-half, head)
  iterations, sized so ScalarE (exp) is 100% busy and the PE interleaves
  head i's scores with head i-1's PV accumulation with no FIFO stalls:
    iter i: scores(i) [PE] + exp(i) PSUM->SBUF fp16 [ACT]
            PV(i-1) accumulation [PE, interleaved per key-block]
            evacuate PV psum + reciprocal of sums (i-1) [DVE]
            1/sums broadcast via DRAM-bounce DMA, normalize mult (i-2) [DVE]
  y_partial[q, e] = sum_c outT[c].T @ wo[c] -> DRAM
"""

import math

import numpy as np

import concourse.bass as bass
import concourse.tile as tile
from concourse import bacc, mybir
from concourse.bass_utils import run_bass_kernel_spmd

F32 = mybir.dt.float32
F32R = mybir.dt.float32r
BF16 = mybir.dt.bfloat16
FP16 = mybir.dt.float16
MM_DT = FP16  # fp16: same speed as bf16, 8x finer mantissa; all values fit fp16 range

B, S, D = 4, 2048, 1024
H, DH = 16, 64
HPC = 8  # heads per core
DPC = HPC * DH  # 512 feature dims per core
SCALE = 1.0 / math.sqrt(DH + 1e-9)

Exp = mybir.ActivationFunctionType.Exp


def _phase1_projections(nc, tc, xT, wq, wk, wv, QT, KT, V):
    NCH = D // 128
    NPAIR = HPC // 2
    with (
        tc.tile_pool(name="xh", bufs=2) as xh,
        tc.tile_pool(name="wt", bufs=18) as wt,
        tc.tile_pool(name="wvt", bufs=8) as wvt,
        tc.tile_pool(name="ps1", bufs=4, space="PSUM") as ps1,
    ):
        wv_tiles = []
        for c in range(NCH):
            w_t = wvt.tile([128, DPC], MM_DT, tag="wv", name=f"wv_{c}")
            nc.sync.dma_start(out=w_t, in_=wv[c * 128 : (c + 1) * 128, :])
            wv_tiles.append(w_t)

        xts = []
        for th in range(2):
            t0 = th * 1024
            xTs = xh.tile([128, NCH, 1024], MM_DT, tag="xT", name=f"xT_{th}")
            engs = [nc.sync, nc.gpsimd, nc.scalar]
            for c in range(NCH):
                engs[c % 3].dma_start(
                    out=xTs[:, c, :], in_=xT[c * 128 : (c + 1) * 128, t0 : t0 + 1024]
                )
            xts.append(xTs)

        def qk_proj(w_d, dst, th, padded, fbs=None):
            t0 = th * 1024
            for fb in (range(NPAIR) if fbs is None else fbs):
                w_tiles = []
                for c in range(NCH):
                    w_t = wt.tile([128, 128], MM_DT, tag="w")
                    nc.sync.dma_start(
                        out=w_t,
                        in_=w_d[c * 128 : (c + 1) * 128, fb * 128 : (fb + 1) * 128],
                    )
                    w_tiles.append(w_t)
                for tb in range(2):
                    pt = ps1.tile([128, 512], F32, tag="ps1")
                    for c in range(NCH):
                        nc.tensor.matmul(
                            pt,
                            w_tiles[c],
                            xts[th][:, c, tb * 512 : (tb + 1) * 512],
                            start=(c == 0),
                            stop=(c == NCH - 1),
                        )
                    tsl = slice(t0 + tb * 512, t0 + (tb + 1) * 512)
                    if padded:
                        # head 2fb -> rows 0:64 of slot 2fb; head 2fb+1 -> rows 64:128 of slot 2fb+1
                        nc.scalar.copy(out=dst[0:64, 2 * fb, tsl], in_=pt[0:64, :])
                        nc.scalar.copy(out=dst[64:128, 2 * fb + 1, tsl], in_=pt[64:128, :])
                    else:
                        nc.scalar.copy(out=dst[:, fb, tsl], in_=pt)

        def v_proj(th):
            for tb in range(8):
                pt = ps1.tile([128, 512], F32, tag="ps1")
                for c in range(NCH):
                    nc.tensor.matmul(
                        pt,
                        xts[th][:, c, tb * 128 : (tb + 1) * 128],
                        wv_tiles[c],
                        start=(c == 0),
                        stop=(c == NCH - 1),
                    )
                nc.scalar.copy(
                    out=V[:, th * 8 + tb, :, 0:64],
                    in_=pt.rearrange("p (h d) -> p h d", h=HPC),
                )

        # K and V (needed in full by phase 2) first, then Q (needed per-pair)
        qk_proj(wk, KT, 0, False)
        qk_proj(wk, KT, 1, False)
        v_proj(0)
        v_proj(1)
        # pair 0's Q first so attention can start; the rest fills PE slack
        qk_proj(wq, QT, 0, True, fbs=[0])
        qk_proj(wq, QT, 1, True, fbs=[0])
        qk_proj(wq, QT, 0, True, fbs=[1, 2, 3])
        qk_proj(wq, QT, 1, True, fbs=[1, 2, 3])


def _phase2_attention(nc, tc, QT, KT, V, outT):
    NPAIR = HPC // 2
    NKB = S // 128  # 16 key blocks
    heads = [(p, qh, h2) for qh in range(2) for p in range(NPAIR) for h2 in range(2)]
    NIT = len(heads)  # 16

    with (
        tc.tile_pool(name="sps", bufs=2, space="PSUM") as sps,
        tc.tile_pool(name="pvs", bufs=2, space="PSUM") as pvs,
        tc.tile_pool(name="ep", bufs=2) as ep,
        tc.tile_pool(name="mp", bufs=2) as mp,
        tc.tile_pool(name="rdp", bufs=3, space="DRAM") as rdp,
    ):
        eth = {}
        pv = {}
        unorm = {}
        rb_sb = {}

        def normalize_start(j):
            # evacuate pv psum, reciprocal of sums, DRAM-bounce broadcast
            unorm[j] = mp.tile([65, 1024], F32, tag="unorm", bufs=3, name=f"unorm_{j}")
            nc.vector.tensor_copy(out=unorm[j], in_=pv[j])
            del pv[j], eth[j]
            srec = mp.tile([65, 1024], F32, tag="srec", name=f"srec_{j}")
            with nc.allow_low_precision(reason="softmax reciprocal"):
                nc.vector.reciprocal(out=srec[64:65, :], in_=unorm[j][64:65, :])
            rd = rdp.tile([1, 1024], F32, name=f"rd_{j}")
            nc.sync.dma_start(out=rd, in_=srec[64:65, :])
            rb_sb[j] = mp.tile([64, 1024], F32, tag="rb", bufs=3, name=f"rb_sb_{j}")
            rd_bcast = bass.AP(tensor=rd.tensor, offset=rd.offset, ap=[[0, 64]] + list(rd.ap[1:]))
            nc.sync.dma_start(out=rb_sb[j], in_=rd_bcast)

        def normalize_finish(j):
            pj, q0j, h2j = heads[j][0], heads[j][1] * 1024, heads[j][2]
            if h2j == 0:
                nc.vector.tensor_mul(
                    out=outT[0:64, pj, q0j : q0j + 1024],
                    in0=unorm[j][0:64, :],
                    in1=rb_sb[j][0:64, :],
                )
            else:
                stg = mp.tile([64, 1024], MM_DT, tag="stg", name=f"stg_{j}")
                nc.vector.tensor_mul(
                    out=stg[0:64, :], in0=unorm[j][0:64, :], in1=rb_sb[j][0:64, :]
                )
                nc.sync.dma_start(
                    out=outT[64:128, pj, q0j : q0j + 1024], in_=stg[0:64, :]
                )
            del unorm[j], rb_sb[j]

        for i in range(NIT + 2):
            cur = heads[i] if i < NIT else None
            if cur is not None:
                p, qh, h2 = cur
                q0 = qh * 1024
                h = p * 2 + h2
                eth[i] = ep.tile([128, NKB, 1024], MM_DT, tag="eth", name=f"eth_{i}")
            for kb in range(NKB):
                if cur is not None:
                    st = sps.tile([128, 1024], F32, tag="s", name=f"st_{i}_{kb}")
                    for qb in range(2):
                        nc.tensor.matmul(
                            st[:, qb * 512 : (qb + 1) * 512],
                            KT[:, p, kb * 128 : (kb + 1) * 128],
                            QT[:, h, q0 + qb * 512 : q0 + (qb + 1) * 512],
                            start=True,
                            stop=True,
                        )
                    nc.scalar.activation(
                        out=eth[i][:, kb, :], in_=st, func=Exp, scale=SCALE
                    )
                if i >= 1 and (i - 1) in eth:
                    pp, pqh, ph2 = heads[i - 1]
                    if kb == 0:
                        pv[i - 1] = pvs.tile(
                            [65, 1024], F32, tag="pv", name=f"pv_{i-1}"
                        )
                    for qb in range(2):
                        nc.tensor.matmul(
                            pv[i - 1][:, qb * 512 : (qb + 1) * 512],
                            V[:, kb, pp * 2 + ph2, :],
                            eth[i - 1][:, kb, qb * 512 : (qb + 1) * 512],
                            start=(kb == 0),
                            stop=(kb == NKB - 1),
                        )
            if i >= 1 and (i - 1) in pv:
                normalize_start(i - 1)
            if i >= 2 and (i - 2) in unorm:
                normalize_finish(i - 2)


def _phase3_output_proj(nc, tc, outT, wo_sb, y):
    NPAIR = HPC // 2
    with (
        tc.tile_pool(name="yps", bufs=6, space="PSUM") as yps,
        tc.tile_pool(name="ysb", bufs=3) as ysb,
    ):
        for qb in range(S // 128):
            y_sb = ysb.tile([128, D], F32, tag="y")
            for eb in range(2):
                yp = yps.tile([128, 512], F32, tag="yp")
                for c in range(NPAIR):
                    nc.tensor.matmul(
                        yp,
                        outT[:, c, qb * 128 : (qb + 1) * 128],
                        wo_sb[:, c, eb * 512 : (eb + 1) * 512],
                        start=(c == 0),
                        stop=(c == NPAIR - 1),
                    )
                if eb == 0:
                    nc.vector.tensor_copy(out=y_sb[:, eb * 512 : (eb + 1) * 512], in_=yp)
                else:
                    nc.scalar.copy(out=y_sb[:, eb * 512 : (eb + 1) * 512], in_=yp)
            yeng = nc.sync if qb % 2 == 0 else nc.gpsimd
            yeng.dma_start(out=y[qb * 128 : (qb + 1) * 128, :], in_=y_sb)


def build_program():
    nc = bacc.Bacc("TRN2", target_bir_lowering=False, debug=False, num_devices=8)

    xT = nc.dram_tensor("xT", [D, S], MM_DT, kind="ExternalInput")
    wq = nc.dram_tensor("wq", [D, DPC], MM_DT, kind="ExternalInput")
    wk = nc.dram_tensor("wk", [D, DPC], MM_DT, kind="ExternalInput")
    wv = nc.dram_tensor("wv", [D, DPC], MM_DT, kind="ExternalInput")
    wo = nc.dram_tensor("wo", [DPC, D], MM_DT, kind="ExternalInput")
    y = nc.dram_tensor("y", [S, D], F32, kind="ExternalOutput")

    NPAIR = HPC // 2

    with tile.TileContext(nc) as tc:
        with (
            tc.tile_pool(name="qkv", bufs=1) as qkv,
            tc.tile_pool(name="consts", bufs=1) as consts,
        ):
            QT = qkv.tile([128, HPC, S], MM_DT, name="QT")
            nc.vector.memset(QT, 0.0)
            KT = qkv.tile([128, NPAIR, S], MM_DT, name="KT")
            V = qkv.tile([128, S // 128, HPC, 65], MM_DT, name="V")
            # ones column of V_aug (cols 0:64 overwritten by projection copies)
            nc.vector.memset(V, 1.0)

            _phase1_projections(nc, tc, xT, wq, wk, wv, QT, KT, V)

            with tc.tile_pool(name="big", bufs=1) as bigpool:
                outT = bigpool.tile([128, NPAIR, S], MM_DT, name="outT")
                wo_sb = bigpool.tile([128, NPAIR, D], MM_DT, name="wo_sb")
                nc.sync.dma_start(out=wo_sb, in_=wo.rearrange("(c p) e -> p c e", p=128))
                _phase2_attention(nc, tc, QT, KT, V, outT)
                _phase3_output_proj(nc, tc, outT, wo_sb, y)

    nc.compile()
    return nc


_program_cache = {}


def _get_program():
    if "nc" not in _program_cache:
        _program_cache["nc"] = build_program()
    return _program_cache["nc"]


def build_in_maps(x, w_qkv, w_o):
    import ml_dtypes

    np_dt = mybir.dt.np(MM_DT)
    in_maps = []
    for c in range(8):
        b, hh = c // 2, c % 2
        f0 = hh * DPC
        in_maps.append(
            {
                "xT": np.ascontiguousarray(x[b].T.astype(np_dt)),
                "wq": np.ascontiguousarray(w_qkv[:, f0 : f0 + DPC].astype(np_dt)),
                "wk": np.ascontiguousarray(w_qkv[:, D + f0 : D + f0 + DPC].astype(np_dt)),
                "wv": np.ascontiguousarray(w_qkv[:, 2 * D + f0 : 2 * D + f0 + DPC].astype(np_dt)),
                "wo": np.ascontiguousarray(w_o[f0 : f0 + DPC, :].astype(np_dt)),
            }
        )
    return in_maps


def kernel(x: np.ndarray, w_qkv: np.ndarray, w_o: np.ndarray) -> np.ndarray:
    x = np.ascontiguousarray(np.asarray(x, dtype=np.float32))
    w_qkv = np.ascontiguousarray(np.asarray(w_qkv, dtype=np.float32))
    w_o = np.ascontiguousarray(np.asarray(w_o, dtype=np.float32))
    assert x.shape == (B, S, D) and w_qkv.shape == (D, 3 * D) and w_o.shape == (D, D)
    nc = _get_program()
    res = run_bass_kernel_spmd(nc, build_in_maps(x, w_qkv, w_o), core_ids=list(range(8)))
    out = np.empty((B, S, D), dtype=np.float32)
    for b in range(B):
        out[b] = res.results[2 * b]["y"] + res.results[2 * b + 1]["y"]
    return out
